# revision 1
# baseline (speedup 1.0000x reference)
"""Multi-head attention (B=4, S=2048, D=1024, H=16) on 8 TRN2 NeuronCores.

Sharding: core c handles batch b = c//2 and query-half qh = c%2 (1024 query
rows), with K/V projection for its batch replicated across the 2 cores that
share the batch. Zero inter-core communication; host just slices inputs and
concatenates outputs.

Per-core dataflow (all matmuls float32r unless noted):
  1. PE-transpose inputs to X^T layout ([d_model on partitions, seq free]).
  2. Projections: Q^T/K^T = W^T chunks @ X^T  (evicted to bf16, +bias),
     V = X^T-chunks(stationary) @ Wv (normal [s, dv] layout, f32r).
  3. Per head-pair, per q-tile(512): scores^T = K_h^T.T @ Q_h^T (bf16 matmul,
     2 heads row-packed in the PE array), exp via ScalarE (scale=1/32) to
     f32r, PV col-packed (2 heads), softmax sums via ones-matmul (M=1),
     normalize O^T with GPSIMD-broadcast reciprocals (+bv).
  4. Final: out = O^T-chunks.T @ Wo + bo (bo added via a K=1 ones matmul).
"""

import numpy as np

import concourse.bacc as bacc
import concourse.mybir as mybir
import concourse.tile as tile
from concourse import bass_utils
from concourse.masks import make_identity

F32 = mybir.dt.float32
F32R = mybir.dt.float32r
BF16 = mybir.dt.bfloat16
EXP = mybir.ActivationFunctionType.Exp
COPY = mybir.ActivationFunctionType.Copy

B, S, D, H = 4, 2048, 1024, 16
SQ = 1024          # query rows per core
P = 128
MC = D // P        # 8 m-chunks (contraction of projections)
DKC = D // P       # 8 dk-chunks
KC = S // P        # 16 key chunks
SCALE = 1.0 / 32.0  # 1/sqrt(D_K)
N_CORES = 8

_CACHED_NC = None


def build_nc():
    nc = bacc.Bacc("TRN2", target_bir_lowering=False, debug=False,
                   num_devices=N_CORES)
    q_in = nc.dram_tensor("q_in", [SQ, D], F32, kind="ExternalInput")
    k_in = nc.dram_tensor("k_in", [S, D], F32, kind="ExternalInput")
    v_in = nc.dram_tensor("v_in", [S, D], F32, kind="ExternalInput")
    wq_d = nc.dram_tensor("wq", [D, D], F32, kind="ExternalInput")
    wk_d = nc.dram_tensor("wk", [D, D], F32, kind="ExternalInput")
    wv_d = nc.dram_tensor("wv", [D, D], F32, kind="ExternalInput")
    wo_d = nc.dram_tensor("wo", [D, D], F32, kind="ExternalInput")
    bq_d = nc.dram_tensor("bq", [D], F32, kind="ExternalInput")
    bk_d = nc.dram_tensor("bk", [D], F32, kind="ExternalInput")
    bv_d = nc.dram_tensor("bv", [D], F32, kind="ExternalInput")
    bo_d = nc.dram_tensor("bo", [D], F32, kind="ExternalInput")
    out_d = nc.dram_tensor("out", [SQ, D], F32, kind="ExternalOutput")

    with tile.TileContext(nc) as tc:
        with tc.tile_pool(name="const", bufs=1) as constp:
            ident = constp.tile([P, P], F32)
            make_identity(nc, ident[:])
            ones_f = constp.tile([P, 1], F32)
            nc.vector.memset(ones_f[:], 1.0)
            ones_col = constp.tile([P, 1], F32R)
            nc.vector.tensor_copy(ones_col[:], ones_f[:])
            onesr_f = constp.tile([1, P], F32)
            nc.vector.memset(onesr_f[:], 1.0)
            ones_row = constp.tile([1, P], F32R)
            nc.vector.tensor_copy(ones_row[:], onesr_f[:])
            bq_t = constp.tile([P, MC], F32)
            nc.sync.dma_start(bq_t[:], bq_d.ap().rearrange("(c p) -> p c", p=P))
            bk_t = constp.tile([P, MC], F32)
            nc.sync.dma_start(bk_t[:], bk_d.ap().rearrange("(c p) -> p c", p=P))
            bv_t = constp.tile([P, MC], F32)
            nc.sync.dma_start(bv_t[:], bv_d.ap().rearrange("(c p) -> p c", p=P))
            bo_f = constp.tile([1, D], F32)
            nc.sync.dma_start(bo_f[:], bo_d.ap().unsqueeze(0))
            bo_t = constp.tile([1, D], F32R)
            nc.vector.tensor_copy(bo_t[:], bo_f[:])

            _build_body(nc, tc, q_in, k_in, v_in, wq_d, wk_d, wv_d, wo_d,
                        bq_t, bk_t, bv_t, bo_t, ident, ones_col, ones_row,
                        out_d)
    nc.compile()
    return nc


def _load_w(nc, wpool, stg, w_d, tag):
    """DMA weight matrix row-chunks and round to f32r. Returns 8 tiles
    [128, D] (f32r), tile mm = rows [128*mm, 128*mm+128)."""
    tiles = []
    for mm in range(MC):
        raw = stg.tile([P, D], F32, tag="wraw")
        nc.sync.dma_start(raw[:], w_d.ap()[mm * P:(mm + 1) * P, :])
        t = wpool.tile([P, D], F32R, tag=f"{tag}{mm}", name=f"wt_{tag}{mm}")
        nc.vector.tensor_copy(t[:], raw[:])
        tiles.append(t)
    return tiles


def _transpose_groups(nc, x_d, n_rows, stg, ps_t, ident, evict):
    """PE-transpose x_d [n_rows, D] in groups of 4 row-chunks.

    For each group g and m-chunk mm, produces a [128, 512] transposed block
    (partitions = m, free = the group's 4x128 seq rows) in PSUM and calls
    evict(mm, g, psum_slice) to store it."""
    ngroups = n_rows // (4 * P)
    for g in range(ngroups):
        rows = []
        for j in range(4):
            r = g * 4 + j
            t = stg.tile([P, D], F32, tag="xin", bufs=6)
            nc.sync.dma_start(t[:], x_d.ap()[r * P:(r + 1) * P, :])
            rows.append(t)
        for mm in range(MC):
            pst = ps_t.tile([P, 512], F32, tag="pst")
            for j in range(4):
                nc.tensor.transpose(
                    pst[:, j * P:(j + 1) * P],
                    rows[j][:, mm * P:(mm + 1) * P], ident[:])
            evict(mm, g, pst)


def _normalize_pair(nc, OT, rp, bcp, bv_t, qs, pair, pv1, pv2):
    """Softmax-normalize both heads of a pair from fused PV psums
    (row 64 = sums) into OT; odd head partition-shifted via DMA.

    The psum is first evicted wholesale to SBUF with one DVE copy so the
    PSUM bank frees fast (keeps the PV accumulation pipeline moving)."""
    F32_, F32R_ = F32, F32R
    for hh, pvp in ((0, pv1), (1, pv2)):
        rb = rp.tile([P, 512], F32_, tag="rb", bufs=1, name="rb")
        nc.vector.tensor_copy(rb[64:65, :], pvp[64:65, :])
        r0 = rp.tile([1, 512], F32_, tag="r0", bufs=2, name="r0")
        nc.gpsimd.tensor_copy(r0[:], rb[64:65, :])
        rr = rp.tile([1, 512], F32_, tag="rr", bufs=2, name="rr")
        nc.vector.reciprocal_approx_fast(rr[:], r0[:])
        bc = bcp.tile([64, 512], F32_, tag="bc", name="bc")
        nc.gpsimd.partition_broadcast(bc[:], rr[:])
        if hh == 0:
            osl = OT[pair][0:64, qs]
            nc.vector.tensor_mul(osl, pvp[0:64, :], bc[:])
            nc.vector.tensor_scalar_add(osl, osl, bv_t[0:64, pair:pair + 1])
        else:
            tmp = bcp.tile([64, 512], F32R_, tag="tmp", bufs=1, name="tmp")
            nc.vector.tensor_mul(tmp[:], pvp[0:64, :], bc[:])
            osl = OT[pair][64:128, qs]
            nc.sync.dma_start(osl, tmp[:])
            nc.vector.tensor_scalar_add(osl, osl, bv_t[64:128, pair:pair + 1])


def _build_body(nc, tc, q_in, k_in, v_in, wq_d, wk_d, wv_d, wo_d,
                bq_t, bk_t, bv_t, bo_t, ident, ones_col, ones_row, out_d):
    # ---------------- persistent pools (LIFO stack) ----------------
    with tc.tile_pool(name="qtp", bufs=1) as qtp:
        QT = [qtp.tile([P, SQ], BF16, tag=f"qt{i}", name=f"qt{i}") for i in range(DKC)]

        # ---- stage Q ----
        with (
            tc.tile_pool(name="xtq", bufs=1) as xtp,
            tc.tile_pool(name="wq", bufs=1) as wpool,
            tc.tile_pool(name="stgq", bufs=2) as stg,
            tc.tile_pool(name="psq_t", bufs=2, space="PSUM") as ps_t,
            tc.tile_pool(name="psq_p", bufs=2, space="PSUM") as ps_p,
        ):
            xqT = [xtp.tile([P, SQ], F32R, tag=f"xt{i}", name=f"xqt{i}") for i in range(MC)]
            wq_t = _load_w(nc, wpool, stg, wq_d, "w")

            def evq(mm, g, pst):
                nc.scalar.activation(
                    xqT[mm][:, g * 512:(g + 1) * 512], pst[:], COPY)
            _transpose_groups(nc, q_in, SQ, stg, ps_t, ident, evq)

            for dk in range(DKC):
                for nh in range(SQ // 512):
                    ps = ps_p.tile([P, 512], F32, tag="pp")
                    for mm in range(MC):
                        nc.tensor.matmul(
                            ps[:], wq_t[mm][:, dk * P:(dk + 1) * P],
                            xqT[mm][:, nh * 512:(nh + 1) * 512],
                            start=(mm == 0), stop=(mm == MC - 1))
                    nc.vector.tensor_scalar_add(
                        QT[dk][:, nh * 512:(nh + 1) * 512], ps[:],
                        bq_t[:, dk:dk + 1])

        with tc.tile_pool(name="ktp", bufs=1) as ktp:
            KT = [ktp.tile([P, S], BF16, tag=f"kt{i}", name=f"kt{i}") for i in range(DKC)]

            # ---- stage K ----
            with (
                tc.tile_pool(name="xtk", bufs=1) as xtp,
                tc.tile_pool(name="wk", bufs=1) as wpool,
                tc.tile_pool(name="stgk", bufs=2) as stg,
                tc.tile_pool(name="psk_t", bufs=2, space="PSUM") as ps_t,
                tc.tile_pool(name="psk_p", bufs=2, space="PSUM") as ps_p,
            ):
                xkT = [xtp.tile([P, S], F32R, tag=f"xt{i}", name=f"xkt{i}") for i in range(MC)]
                wk_t = _load_w(nc, wpool, stg, wk_d, "w")

                def evk(mm, g, pst):
                    nc.scalar.activation(
                        xkT[mm][:, g * 512:(g + 1) * 512], pst[:], COPY)
                _transpose_groups(nc, k_in, S, stg, ps_t, ident, evk)

                for dk in range(DKC):
                    for nh in range(S // 512):
                        ps = ps_p.tile([P, 512], F32, tag="pp")
                        for mm in range(MC):
                            nc.tensor.matmul(
                                ps[:], wk_t[mm][:, dk * P:(dk + 1) * P],
                                xkT[mm][:, nh * 512:(nh + 1) * 512],
                                start=(mm == 0), stop=(mm == MC - 1))
                        nc.vector.tensor_scalar_add(
                            KT[dk][:, nh * 512:(nh + 1) * 512], ps[:],
                            bk_t[:, dk:dk + 1])

            with tc.tile_pool(name="vp", bufs=1) as vp:
                DEXT = H * 65  # V_ext: 65 cols per head (64 V + ones)
                V = [vp.tile([P, DEXT], F32R, tag=f"v{i}", name=f"v{i}")
                     for i in range(KC)]

                # ---- stage V ----
                with (
                    tc.tile_pool(name="vtt", bufs=1) as vtt,
                    tc.tile_pool(name="wv", bufs=1) as wpool,
                    tc.tile_pool(name="stgv", bufs=2) as stg,
                    tc.tile_pool(name="psv_t", bufs=2, space="PSUM") as ps_t,
                    tc.tile_pool(name="psv_p", bufs=2, space="PSUM") as ps_p,
                ):
                    wv_t = _load_w(nc, wpool, stg, wv_d, "w")
                    valT = [vtt.tile([P, 512], F32R, tag=f"vt{i}", name=f"vt{i}")
                            for i in range(MC)]
                    ones16 = vtt.tile([P, H], F32, name="ones16")
                    nc.vector.memset(ones16[:], 1.0)

                    ngroups = S // (4 * P)
                    for g in range(ngroups):
                        rows = []
                        for j in range(4):
                            r = g * 4 + j
                            t = stg.tile([P, D], F32, tag="xin", bufs=6)
                            nc.sync.dma_start(t[:], v_in.ap()[r * P:(r + 1) * P, :])
                            rows.append(t)
                        for mm in range(MC):
                            pst = ps_t.tile([P, 512], F32, tag="pst")
                            for j in range(4):
                                nc.tensor.transpose(
                                    pst[:, j * P:(j + 1) * P],
                                    rows[j][:, mm * P:(mm + 1) * P], ident[:])
                            nc.scalar.activation(valT[mm][:], pst[:], COPY)
                        for j in range(4):
                            sc = g * 4 + j
                            vx = V[sc].rearrange("p (h c) -> p h c", c=65)
                            nc.vector.tensor_copy(
                                vx[:, :, 64:65],
                                ones16[:].rearrange("p (h c) -> p h c", c=1))
                            for nh in range(2):
                                ps = ps_p.tile([P, 512], F32, tag="pp")
                                for mm in range(MC):
                                    nc.tensor.matmul(
                                        ps[:], valT[mm][:, j * P:(j + 1) * P],
                                        wv_t[mm][:, nh * 512:(nh + 1) * 512],
                                        start=(mm == 0), stop=(mm == MC - 1))
                                nc.vector.tensor_copy(
                                    vx[:, 8 * nh:8 * nh + 8, 0:64],
                                    ps[:].rearrange("p (h c) -> p h c", c=64))

                with tc.tile_pool(name="otp", bufs=1) as otp:
                    OT = [otp.tile([P, SQ], F32R, tag=f"ot{i}", name=f"ot{i}")
                          for i in range(DKC)]

                    # ---- attention + final ----
                    with (
                        tc.tile_pool(name="ep", bufs=4) as ep,
                        tc.tile_pool(name="bcp", bufs=2) as bcp,
                        tc.tile_pool(name="rp", bufs=3) as rp,
                        tc.tile_pool(name="ps_sc", bufs=2, space="PSUM") as ps_sc,
                        tc.tile_pool(name="ps_pv", bufs=4, space="PSUM") as ps_pv,
                        tc.tile_pool(name="wo", bufs=1) as wop,
                        tc.tile_pool(name="fin", bufs=2) as finp,
                    ):
                        for qt in range(SQ // 512):
                            qs = slice(qt * 512, (qt + 1) * 512)
                            for pair in range(H // 2):
                                pv1 = ps_pv.tile([P, 512], F32, tag="pv")
                                pv2 = ps_pv.tile([P, 512], F32, tag="pv")
                                c1 = (2 * pair) * 65
                                c2 = (2 * pair + 1) * 65
                                for k2 in range(KC // 2):
                                    ka = slice(2 * k2 * P, (2 * k2 + 1) * P)
                                    kb = slice((2 * k2 + 1) * P,
                                               (2 * k2 + 2) * P)
                                    s1 = ps_sc.tile([P, 1024], F32, tag="sc")
                                    s2 = ps_sc.tile([P, 1024], F32, tag="sc")
                                    nc.tensor.matmul(
                                        s1[:, 0:512], KT[pair][0:64, ka],
                                        QT[pair][0:64, qs],
                                        start=True, stop=True,
                                        tile_position=(0, 0))
                                    nc.tensor.matmul(
                                        s2[:, 0:512], KT[pair][64:128, ka],
                                        QT[pair][64:128, qs],
                                        start=True, stop=True,
                                        tile_position=(64, 0))
                                    nc.tensor.matmul(
                                        s1[:, 512:1024], KT[pair][0:64, kb],
                                        QT[pair][0:64, qs],
                                        start=True, stop=True,
                                        tile_position=(0, 0))
                                    nc.tensor.matmul(
                                        s2[:, 512:1024], KT[pair][64:128, kb],
                                        QT[pair][64:128, qs],
                                        start=True, stop=True,
                                        tile_position=(64, 0))
                                    e1 = ep.tile([P, 1024], F32R, tag="e")
                                    e2 = ep.tile([P, 1024], F32R, tag="e")
                                    nc.scalar.activation(e1[:], s1[:], EXP,
                                                         scale=SCALE)
                                    nc.scalar.activation(e2[:], s2[:], EXP,
                                                         scale=SCALE)
                                    first = k2 == 0
                                    last = k2 == KC // 2 - 1
                                    nc.tensor.matmul(
                                        pv1[0:65, :],
                                        V[2 * k2][:, c1:c1 + 65],
                                        e1[:, 0:512], start=first,
                                        stop=False)
                                    nc.tensor.matmul(
                                        pv2[0:65, :],
                                        V[2 * k2][:, c2:c2 + 65],
                                        e2[:, 0:512], start=first,
                                        stop=False)
                                    nc.tensor.matmul(
                                        pv1[0:65, :],
                                        V[2 * k2 + 1][:, c1:c1 + 65],
                                        e1[:, 512:1024], start=False,
                                        stop=last)
                                    nc.tensor.matmul(
                                        pv2[0:65, :],
                                        V[2 * k2 + 1][:, c2:c2 + 65],
                                        e2[:, 512:1024], start=False,
                                        stop=last)
                                _normalize_pair(nc, OT, rp, bcp, bv_t,
                                                qs, pair, pv1, pv2)

                            # final projection for this q-tile's s-chunks
                            for nh in range(2):
                                ns = slice(nh * 512, (nh + 1) * 512)
                                wo_h = []
                                for dk in range(DKC):
                                    raw = finp.tile([P, 512], F32, tag="wraw", bufs=1)
                                    nc.sync.dma_start(
                                        raw[:], wo_d.ap()[dk * P:(dk + 1) * P, ns])
                                    wt = wop.tile([P, 512], F32R,
                                                  tag=f"woh{dk}", name=f"woh{dk}")
                                    nc.vector.tensor_copy(wt[:], raw[:])
                                    wo_h.append(wt)
                                for sc in range(qt * 4, (qt + 1) * 4):
                                    ss = slice(sc * P, (sc + 1) * P)
                                    fps = ps_pv.tile([P, 512], F32, tag="pv")
                                    for dk in range(DKC):
                                        nc.tensor.matmul(
                                            fps[:], OT[dk][:, ss],
                                            wo_h[dk][:],
                                            start=(dk == 0), stop=False)
                                    nc.tensor.matmul(
                                        fps[:], ones_row[:], bo_t[:, ns],
                                        start=False, stop=True)
                                    ob = finp.tile([P, 512], F32, tag="ob", bufs=1)
                                    nc.vector.tensor_copy(ob[:], fps[:])
                                    nc.sync.dma_start(out_d.ap()[ss, ns], ob[:])


def get_nc():
    global _CACHED_NC
    if _CACHED_NC is None:
        _CACHED_NC = build_nc()
    return _CACHED_NC


def run(inputs, **kwargs):
    """Run on 8 cores; returns (full_output, BassKernelResults)."""
    nc = get_nc()
    queries = np.ascontiguousarray(np.asarray(inputs["queries"], np.float32))
    keys = np.ascontiguousarray(np.asarray(inputs["keys"], np.float32))
    values = np.ascontiguousarray(np.asarray(inputs["values"], np.float32))
    base = {
        "wq": np.ascontiguousarray(np.asarray(inputs["Wq"], np.float32)),
        "wk": np.ascontiguousarray(np.asarray(inputs["Wk"], np.float32)),
        "wv": np.ascontiguousarray(np.asarray(inputs["Wv"], np.float32)),
        "wo": np.ascontiguousarray(np.asarray(inputs["Wo"], np.float32)),
        "bq": np.ascontiguousarray(np.asarray(inputs["bq"], np.float32)),
        "bk": np.ascontiguousarray(np.asarray(inputs["bk"], np.float32)),
        "bv": np.ascontiguousarray(np.asarray(inputs["bv"], np.float32)),
        "bo": np.ascontiguousarray(np.asarray(inputs["bo"], np.float32)),
    }
    in_maps = []
    for c in range(N_CORES):
        b, qh = c // 2, c % 2
        m = dict(base)
        m["q_in"] = np.ascontiguousarray(queries[b, qh * SQ:(qh + 1) * SQ])
        m["k_in"] = keys[b]
        m["v_in"] = values[b]
        in_maps.append(m)
    res = bass_utils.run_bass_kernel_spmd(
        nc, in_maps, core_ids=list(range(N_CORES)), **kwargs)
    out = np.empty((B, S, D), np.float32)
    for c in range(N_CORES):
        b, qh = c // 2, c % 2
        out[b, qh * SQ:(qh + 1) * SQ] = res.results[c]["out"]
    return out, res


def kernel(**inputs):
    out, _ = run(inputs)
    return out


if __name__ == "__main__":
    rng = np.random.default_rng(0)
    ins = {
        "queries": rng.standard_normal((B, S, D), dtype=np.float32),
        "keys": rng.standard_normal((B, S, D), dtype=np.float32),
        "values": rng.standard_normal((B, S, D), dtype=np.float32),
        "Wq": (rng.standard_normal((D, D), dtype=np.float32) / 32),
        "bq": np.zeros(D, np.float32),
        "Wk": (rng.standard_normal((D, D), dtype=np.float32) / 32),
        "bk": np.zeros(D, np.float32),
        "Wv": (rng.standard_normal((D, D), dtype=np.float32) / 32),
        "bv": np.zeros(D, np.float32),
        "Wo": (rng.standard_normal((D, D), dtype=np.float32) / 32),
        "bo": np.zeros(D, np.float32),
    }
    out = kernel(**ins)
    print("out", out.shape, out.dtype, np.abs(out).mean())



# revision 9
# speedup vs baseline: 1.0966x; 1.0966x over previous
"""Multi-head attention (B=4, S=2048, D=1024, H=16) on 8 TRN2 NeuronCores.

Sharding: core c handles batch b = c//2 and query-half qh = c%2 (1024 query
rows), with K/V projection for its batch replicated across the 2 cores that
share the batch. Zero inter-core communication; host just slices inputs and
concatenates outputs.

Per-core dataflow (all matmuls bf16; transposes f32r):
  1. PE-transpose inputs to X^T layout, evict psum->bf16.
  2. Projections Q^T/K^T = W^T chunks @ X^T (bf16, +bias on DVE eviction).
  3. Attention main loop (qt, pair, k2), software-pipelined: scores(k2)
     issued before PV(k2-1) so the PE never stalls on the exp; V-projection
     for later head-pairs and the final output projection for finished
     q-tiles are interleaved as PE filler so the PE stays dense (keeps the
     HAM clock-gate warm).
  4. Softmax sums ride the PV matmul as a 65th V column; normalization on
     DVE/GPSIMD with a fast psum eviction so PSUM banks recycle quickly.
  5. Final: out = O^T-chunks.T @ Wo + bo (bo via pre-broadcast DVE add).
"""

import numpy as np

import concourse.bacc as bacc
import concourse.mybir as mybir
import concourse.tile as tile
from concourse import bass_utils
from concourse.masks import make_identity

F32 = mybir.dt.float32
F32R = mybir.dt.float32r
BF16 = mybir.dt.bfloat16
EXP = mybir.ActivationFunctionType.Exp
COPY = mybir.ActivationFunctionType.Copy

B, S, D, H = 4, 2048, 1024, 16
SQ = 1024          # query rows per core
P = 128
MC = D // P        # 8 m-chunks (contraction of projections)
DKC = D // P       # 8 dk-chunks
KC = S // P        # 16 key chunks
SCALE = 1.0 / 32.0  # 1/sqrt(D_K)
N_CORES = 8

_CACHED_NC = None


def build_nc():
    nc = bacc.Bacc("TRN2", target_bir_lowering=False, debug=False,
                   num_devices=N_CORES)
    q_in = nc.dram_tensor("q_in", [SQ, D], F32, kind="ExternalInput")
    k_in = nc.dram_tensor("k_in", [S, D], F32, kind="ExternalInput")
    v_in = nc.dram_tensor("v_in", [S, D], F32, kind="ExternalInput")
    wq_d = nc.dram_tensor("wq", [D, D], F32, kind="ExternalInput")
    wk_d = nc.dram_tensor("wk", [D, D], F32, kind="ExternalInput")
    wv_d = nc.dram_tensor("wv", [D, D], F32, kind="ExternalInput")
    wo_d = nc.dram_tensor("wo", [D, D], F32, kind="ExternalInput")
    bq_d = nc.dram_tensor("bq", [D], F32, kind="ExternalInput")
    bk_d = nc.dram_tensor("bk", [D], F32, kind="ExternalInput")
    bv_d = nc.dram_tensor("bv", [D], F32, kind="ExternalInput")
    bo_d = nc.dram_tensor("bo", [D], F32, kind="ExternalInput")
    out_d = nc.dram_tensor("out", [SQ, D], F32, kind="ExternalOutput")

    with tile.TileContext(nc) as tc:
        _build_body(nc, tc, q_in, k_in, v_in, wq_d, wk_d, wv_d, wo_d,
                    bq_d, bk_d, bv_d, bo_d, out_d)
    nc.compile()
    return nc


def _transpose_stage(nc, x_d, n_rows, stg, ps_t, identr, dma_engines, evict):
    """DMA x_d row-chunks (alternating queues) and PE-transpose.
    evict(mm, g, psum_slice) stores each [128, 512] transposed block."""
    ngroups = n_rows // (4 * P)
    for g in range(ngroups):
        rows = []
        for j in range(4):
            r = g * 4 + j
            t = stg.tile([P, D], F32, tag="xin", bufs=6)
            dma_engines[r % 2].dma_start(t[:], x_d.ap()[r * P:(r + 1) * P, :])
            rows.append(t)
        for mm in range(MC):
            pst = ps_t.tile([P, 512], F32, tag="pst")
            for j in range(4):
                nc.tensor.transpose(
                    pst[:, j * P:(j + 1) * P],
                    rows[j][:, mm * P:(mm + 1) * P], identr[:])
            evict(mm, g, pst)


def _load_w_bf16(nc, wpool, stg, w_d, tag, dma_engines):
    """DMA weight row-chunks (alternating queues) and cast to bf16."""
    tiles = []
    for mm in range(MC):
        raw = stg.tile([P, D], F32, tag="wraw", bufs=2)
        dma_engines[mm % 2].dma_start(raw[:], w_d.ap()[mm * P:(mm + 1) * P, :])
        t = wpool.tile([P, D], BF16, tag=f"{tag}{mm}", name=f"wt_{tag}{mm}")
        nc.vector.tensor_copy(t[:], raw[:])
        tiles.append(t)
    return tiles


def _build_body(nc, tc, q_in, k_in, v_in, wq_d, wk_d, wv_d, wo_d,
                bq_d, bk_d, bv_d, bo_d, out_d):
    dmae = [nc.sync, nc.scalar]   # the two hwdge queues
    with (
        tc.tile_pool(name="const", bufs=1) as constp,
        tc.tile_pool(name="qtp", bufs=1) as qtp,
        tc.tile_pool(name="ktp", bufs=1) as ktp,
    ):
        ident = constp.tile([P, P], F32)
        make_identity(nc, ident[:])
        identr = ident
        bq_t = constp.tile([P, MC], F32)
        nc.sync.dma_start(bq_t[:], bq_d.ap().rearrange("(c p) -> p c", p=P))
        bk_t = constp.tile([P, MC], F32)
        nc.sync.dma_start(bk_t[:], bk_d.ap().rearrange("(c p) -> p c", p=P))
        bv_t = constp.tile([P, MC], F32)
        nc.sync.dma_start(bv_t[:], bv_d.ap().rearrange("(c p) -> p c", p=P))
        bo_f = constp.tile([1, D], F32)
        nc.sync.dma_start(bo_f[:], bo_d.ap().unsqueeze(0))

        QT = [qtp.tile([P, SQ], BF16, tag=f"qt{i}", name=f"qt{i}")
              for i in range(DKC)]
        KT = [ktp.tile([P, S], BF16, tag=f"kt{i}", name=f"kt{i}")
              for i in range(DKC)]

        # ---------------- stage Q ----------------
        with (
            tc.tile_pool(name="stgq", bufs=1) as stg,
            tc.tile_pool(name="wq", bufs=1) as wpool,
            tc.tile_pool(name="xtq", bufs=1) as xtp,
            tc.tile_pool(name="psq_t", bufs=2, space="PSUM") as ps_t,
            tc.tile_pool(name="psq_p", bufs=2, space="PSUM") as ps_p,
        ):
            xqT = [xtp.tile([P, SQ], BF16, tag=f"xt{i}", name=f"xqt{i}")
                   for i in range(MC)]

            def evq(mm, g, pst):
                nc.scalar.activation(
                    xqT[mm][:, g * 512:(g + 1) * 512], pst[:], COPY)
            _transpose_stage(nc, q_in, SQ, stg, ps_t, identr, dmae, evq)
            wq_t = _load_w_bf16(nc, wpool, stg, wq_d, "w", dmae)

            for dk in range(DKC):
                for nh in range(SQ // 512):
                    ps = ps_p.tile([P, 512], F32, tag="pp")
                    for mm in range(MC):
                        nc.tensor.matmul(
                            ps[:], wq_t[mm][:, dk * P:(dk + 1) * P],
                            xqT[mm][:, nh * 512:(nh + 1) * 512],
                            start=(mm == 0), stop=(mm == MC - 1))
                    nc.vector.tensor_scalar_add(
                        QT[dk][:, nh * 512:(nh + 1) * 512], ps[:],
                        bq_t[:, dk:dk + 1])

        # ---------------- stage K ----------------
        with (
            tc.tile_pool(name="stgk", bufs=1) as stg,
            tc.tile_pool(name="wk", bufs=1) as wpool,
            tc.tile_pool(name="xtk", bufs=1) as xtp,
            tc.tile_pool(name="psk_t", bufs=2, space="PSUM") as ps_t,
            tc.tile_pool(name="psk_p", bufs=2, space="PSUM") as ps_p,
        ):
            xkT = [xtp.tile([P, S], BF16, tag=f"xt{i}", name=f"xkt{i}")
                   for i in range(MC)]

            def evk(mm, g, pst):
                nc.scalar.activation(
                    xkT[mm][:, g * 512:(g + 1) * 512], pst[:], COPY)
            _transpose_stage(nc, k_in, S, stg, ps_t, identr, dmae, evk)
            wk_t = _load_w_bf16(nc, wpool, stg, wk_d, "w", dmae)

            for dk in range(DKC):
                for nh in range(S // 512):
                    ps = ps_p.tile([P, 512], F32, tag="pp")
                    for mm in range(MC):
                        nc.tensor.matmul(
                            ps[:], wk_t[mm][:, dk * P:(dk + 1) * P],
                            xkT[mm][:, nh * 512:(nh + 1) * 512],
                            start=(mm == 0), stop=(mm == MC - 1))
                    nc.vector.tensor_scalar_add(
                        KT[dk][:, nh * 512:(nh + 1) * 512], ps[:],
                        bk_t[:, dk:dk + 1])

        # -------- persistent pools for V / attention / final --------
        DEXT = H * 65  # V_ext: 65 cols per head (64 V + ones)
        with (
            tc.tile_pool(name="xtv", bufs=1) as xvtp,
            tc.tile_pool(name="wv", bufs=1) as wvp,
            tc.tile_pool(name="vp", bufs=1) as vp,
            tc.tile_pool(name="otp", bufs=1) as otp,
        ):
            xvT = [xvtp.tile([P, S], BF16, tag=f"xt{i}", name=f"xvt{i}")
                   for i in range(MC)]
            V = [vp.tile([P, DEXT], BF16, tag=f"v{i}", name=f"v{i}")
                 for i in range(KC)]
            OT = [otp.tile([P, SQ], BF16, tag=f"ot{i}", name=f"ot{i}")
                  for i in range(DKC)]
            ones16 = constp.tile([P, H], BF16, name="ones16")
            nc.vector.memset(ones16[:], 1.0)

            # ---- stage V transposes + weight load ----
            with (
                tc.tile_pool(name="stgv", bufs=1) as stg,
                tc.tile_pool(name="psv_t", bufs=2, space="PSUM") as ps_t,
            ):
                def evv(mm, g, pst):
                    nc.scalar.activation(
                        xvT[mm][:, g * 512:(g + 1) * 512], pst[:], COPY)
                _transpose_stage(nc, v_in, S, stg, ps_t, identr, dmae, evv)
                wv_t = _load_w_bf16(nc, wvp, stg, wv_d, "w", dmae)

            # ---- attention + interleaved V-proj / final ----
            with (
                tc.tile_pool(name="wo", bufs=1) as wop,
                tc.tile_pool(name="ep", bufs=4) as ep,
                tc.tile_pool(name="rp", bufs=1) as rp,
                tc.tile_pool(name="bcp", bufs=1) as bcp,
                tc.tile_pool(name="fin", bufs=1) as finp,
                tc.tile_pool(name="ps_s", bufs=2, space="PSUM") as ps_s,
                tc.tile_pool(name="ps_pv", bufs=3, space="PSUM") as ps_pv,
                tc.tile_pool(name="ps_m", bufs=1, space="PSUM") as ps_m,
            ):
                bo_bc = constp.tile([P, D], F32, name="bo_bc")
                nc.gpsimd.partition_broadcast(bo_bc[:, 0:512], bo_f[:, 0:512])
                nc.gpsimd.partition_broadcast(bo_bc[:, 512:1024],
                                              bo_f[:, 512:1024])

                def vproj_group(sc, nh):
                    ps = ps_m.tile([P, 512], F32, tag="m")
                    for mm in range(MC):
                        nc.tensor.matmul(
                            ps[:], xvT[mm][:, sc * P:(sc + 1) * P],
                            wv_t[mm][:, nh * 512:(nh + 1) * 512],
                            start=(mm == 0), stop=(mm == MC - 1))
                    vx = V[sc].rearrange("p (h c) -> p h c", c=65)
                    if nh == 0:
                        nc.vector.tensor_copy(
                            vx[:, :, 64:65],
                            ones16[:].rearrange("p (h c) -> p h c", c=1))
                    nc.vector.tensor_copy(
                        vx[:, 8 * nh:8 * nh + 8, 0:64],
                        ps[:].rearrange("p (h c) -> p h c", c=64))

                wo_t = {}

                def wo_load(nh, dk):
                    raw = finp.tile([P, 512], F32, tag="wraw", bufs=1)
                    nc.scalar.dma_start(
                        raw[:],
                        wo_d.ap()[dk * P:(dk + 1) * P,
                                  nh * 512:(nh + 1) * 512])
                    wt = wop.tile([P, 512], BF16, tag=f"woh{nh}_{dk}",
                                  name=f"woh{nh}_{dk}")
                    nc.vector.tensor_copy(wt[:], raw[:])
                    wo_t[(nh, dk)] = wt

                def final_group(qt, nh, sc):
                    ss = slice(sc * P, (sc + 1) * P)
                    ns = slice(nh * 512, (nh + 1) * 512)
                    fps = ps_m.tile([P, 512], F32, tag="m")
                    for dk in range(DKC):
                        nc.tensor.matmul(
                            fps[:], OT[dk][:, ss], wo_t[(nh, dk)][:],
                            start=(dk == 0), stop=(dk == DKC - 1))
                    ob = finp.tile([P, 512], F32, tag="ob", bufs=2)
                    nc.vector.tensor_add(ob[:], fps[:], bo_bc[:, ns])
                    nc.sync.dma_start(out_d.ap()[ss, ns], ob[:])

                def make_norm(qs, pair, pv1, pv2):
                    def emit():
                        for hh, pvp in ((0, pv1), (1, pv2)):
                            psb = rp.tile([65, 512], F32, tag="psb", bufs=3,
                                          name="psb")
                            nc.vector.tensor_copy(psb[:], pvp[0:65, :])
                            sums = rp.tile([1, 512], F32, tag="sums", bufs=2,
                                           name="sums")
                            nc.gpsimd.tensor_copy(sums[:], psb[64:65, :])
                            nc.vector.reciprocal_approx_fast(sums[:], sums[:])
                            bc = bcp.tile([64, 512], F32, tag="bc", bufs=2,
                                          name="bc")
                            nc.gpsimd.partition_broadcast(bc[:], sums[:])
                            if hh == 0:
                                osl = OT[pair][0:64, qs]
                                nc.vector.tensor_mul(osl, psb[0:64, :], bc[:])
                                nc.vector.tensor_scalar_add(
                                    osl, osl, bv_t[0:64, pair:pair + 1])
                            else:
                                tmp = bcp.tile([64, 512], BF16, tag="tmp",
                                               bufs=2, name="tmp")
                                nc.vector.tensor_mul(tmp[:], psb[0:64, :],
                                                     bc[:])
                                osl = OT[pair][64:128, qs]
                                nc.sync.dma_start(osl, tmp[:])
                                nc.vector.tensor_scalar_add(
                                    osl, osl, bv_t[64:128, pair:pair + 1])
                    return emit

                def make_pv(pv1, pv2, e1, e2, c1, c2, k2):
                    first = k2 == 0
                    last = k2 == KC // 2 - 1

                    def emit():
                        nc.tensor.matmul(
                            pv1[0:65, :], V[2 * k2][:, c1:c1 + 65],
                            e1[:, 0:512], start=first, stop=False)
                        nc.tensor.matmul(
                            pv2[0:65, :], V[2 * k2][:, c2:c2 + 65],
                            e2[:, 0:512], start=first, stop=False)
                        nc.tensor.matmul(
                            pv1[0:65, :], V[2 * k2 + 1][:, c1:c1 + 65],
                            e1[:, 512:1024], start=False, stop=last)
                        nc.tensor.matmul(
                            pv2[0:65, :], V[2 * k2 + 1][:, c2:c2 + 65],
                            e2[:, 512:1024], start=False, stop=last)
                    return emit

                # ---- filler schedule (PE work interleaved into attention) --
                # V nh=0, sc 0..3 must precede the loop; emitted below.
                fillers = {}
                for it in range(6):     # V nh0 sc 4..15, 2 per iter
                    fillers[it] = [
                        (lambda s=4 + 2 * it + j: vproj_group(s, 0))
                        for j in range(2)]
                for it in range(16):    # V nh1 sc 0..15, 1 per iter
                    fillers.setdefault(8 + it, []).append(
                        lambda s=it: vproj_group(s, 1))
                for i in range(16):     # wo loads (no PE work)
                    nh, dk = divmod(i, 8)
                    fillers.setdefault(26 + i, []).append(
                        lambda n=nh, d=dk: wo_load(n, d))
                for i in range(8):      # final for qt0 during qt1 attention
                    nh, sc = divmod(i, 4)
                    fillers.setdefault(68 + 3 * i, []).append(
                        lambda n=nh, s=sc: final_group(0, n, s))

                # V nh0 for the first key chunks before attention starts
                for sc in range(4):
                    vproj_group(sc, 0)

                pend_pv = None
                pend_norm = None
                it = 0
                for qt in range(SQ // 512):
                    qs = slice(qt * 512, (qt + 1) * 512)
                    for pair in range(H // 2):
                        pv1 = ps_pv.tile([P, 512], F32, tag="pv")
                        pv2 = ps_pv.tile([P, 512], F32, tag="pv")
                        c1 = (2 * pair) * 65
                        c2 = (2 * pair + 1) * 65
                        for k2 in range(KC // 2):
                            ka = slice(2 * k2 * P, (2 * k2 + 1) * P)
                            kb = slice((2 * k2 + 1) * P, (2 * k2 + 2) * P)
                            s1 = ps_s.tile([P, 1024], F32, tag="sc")
                            s2 = ps_s.tile([P, 1024], F32, tag="sc")
                            nc.tensor.matmul(
                                s1[:, 0:512], KT[pair][0:64, ka],
                                QT[pair][0:64, qs], start=True, stop=True,
                                tile_position=(0, 0))
                            nc.tensor.matmul(
                                s2[:, 0:512], KT[pair][64:128, ka],
                                QT[pair][64:128, qs], start=True, stop=True,
                                tile_position=(64, 0))
                            nc.tensor.matmul(
                                s1[:, 512:1024], KT[pair][0:64, kb],
                                QT[pair][0:64, qs], start=True, stop=True,
                                tile_position=(0, 0))
                            nc.tensor.matmul(
                                s2[:, 512:1024], KT[pair][64:128, kb],
                                QT[pair][64:128, qs], start=True, stop=True,
                                tile_position=(64, 0))
                            e1 = ep.tile([P, 1024], BF16, tag="e")
                            e2 = ep.tile([P, 1024], BF16, tag="e")
                            nc.scalar.activation(e1[:], s1[:], EXP,
                                                 scale=SCALE)
                            nc.scalar.activation(e2[:], s2[:], EXP,
                                                 scale=SCALE)
                            if pend_pv is not None:
                                pend_pv()
                                pend_pv = None
                            if pend_norm is not None:
                                # after the prev pair's last PV (flushed at
                                # k2==0 above), before its psum bufs rotate
                                # into reuse by this pair's PV
                                pend_norm()
                                pend_norm = None
                            pend_pv = make_pv(pv1, pv2, e1, e2, c1, c2, k2)
                            if k2 == KC // 2 - 1:
                                # norm closure runs after pv(7) is flushed
                                pend_norm_next = make_norm(qs, pair, pv1, pv2)
                            for f in fillers.get(it, ()):
                                f()
                            it += 1
                        pend_norm = pend_norm_next
                # drain the pipeline
                if pend_pv is not None:
                    pend_pv()
                if pend_norm is not None:
                    pend_norm()
                # final projection for qt=1
                for nh in range(2):
                    for sc in range(4, 8):
                        final_group(1, nh, sc)


def get_nc():
    global _CACHED_NC
    if _CACHED_NC is None:
        _CACHED_NC = build_nc()
    return _CACHED_NC


def run(inputs, **kwargs):
    """Run on 8 cores; returns (full_output, BassKernelResults)."""
    nc = get_nc()
    queries = np.ascontiguousarray(np.asarray(inputs["queries"], np.float32))
    keys = np.ascontiguousarray(np.asarray(inputs["keys"], np.float32))
    values = np.ascontiguousarray(np.asarray(inputs["values"], np.float32))
    base = {
        "wq": np.ascontiguousarray(np.asarray(inputs["Wq"], np.float32)),
        "wk": np.ascontiguousarray(np.asarray(inputs["Wk"], np.float32)),
        "wv": np.ascontiguousarray(np.asarray(inputs["Wv"], np.float32)),
        "wo": np.ascontiguousarray(np.asarray(inputs["Wo"], np.float32)),
        "bq": np.ascontiguousarray(np.asarray(inputs["bq"], np.float32)),
        "bk": np.ascontiguousarray(np.asarray(inputs["bk"], np.float32)),
        "bv": np.ascontiguousarray(np.asarray(inputs["bv"], np.float32)),
        "bo": np.ascontiguousarray(np.asarray(inputs["bo"], np.float32)),
    }
    in_maps = []
    for c in range(N_CORES):
        b, qh = c // 2, c % 2
        m = dict(base)
        m["q_in"] = np.ascontiguousarray(queries[b, qh * SQ:(qh + 1) * SQ])
        m["k_in"] = keys[b]
        m["v_in"] = values[b]
        in_maps.append(m)
    res = bass_utils.run_bass_kernel_spmd(
        nc, in_maps, core_ids=list(range(N_CORES)), **kwargs)
    out = np.empty((B, S, D), np.float32)
    for c in range(N_CORES):
        b, qh = c // 2, c % 2
        out[b, qh * SQ:(qh + 1) * SQ] = res.results[c]["out"]
    return out, res


def kernel(**inputs):
    out, _ = run(inputs)
    return out


if __name__ == "__main__":
    rng = np.random.default_rng(0)
    ins = {
        "queries": rng.standard_normal((B, S, D), dtype=np.float32),
        "keys": rng.standard_normal((B, S, D), dtype=np.float32),
        "values": rng.standard_normal((B, S, D), dtype=np.float32),
        "Wq": (rng.standard_normal((D, D), dtype=np.float32) / 32),
        "bq": np.zeros(D, np.float32),
        "Wk": (rng.standard_normal((D, D), dtype=np.float32) / 32),
        "bk": np.zeros(D, np.float32),
        "Wv": (rng.standard_normal((D, D), dtype=np.float32) / 32),
        "bv": np.zeros(D, np.float32),
        "Wo": (rng.standard_normal((D, D), dtype=np.float32) / 32),
        "bo": np.zeros(D, np.float32),
    }
    out = kernel(**ins)
    print("out", out.shape, out.dtype, np.abs(out).mean())


# revision 14
# speedup vs baseline: 1.1549x; 1.0532x over previous
"""Multi-head attention (B=4, S=2048, D=1024, H=16) on 8 TRN2 NeuronCores.

Sharding: core c handles batch b = c//2 and query-half qh = c%2 (1024 query
rows), with K/V projection for its batch replicated across the 2 cores that
share the batch. Zero inter-core communication; host just slices inputs and
concatenates outputs.

Per-core dataflow (all matmuls and transposes bf16):
  1. Head stages (Q, K, V): per 512-row group, DMA -> DVE cast to bf16 ->
     PE transpose -> projection matmuls, interleaved so the PE stays dense
     behind the DMA stream (keeps the HAM clock-gate warm).
  2. Attention main loop over interleaved (qt, pair) blocks
     [A0 A1 B0 A2 B1 ... A7 B6 B7], software-pipelined: scores(k2) issued
     before PV(k2-1) so the PE never stalls on the exp; V-projection for
     heads 8-15, Wo loads and the final projection for the finished q-half
     are spread as PE filler across all iterations.
  3. Softmax sums ride the PV matmul as a 65th V column; normalization on
     DVE/GPSIMD with a fast psum eviction so PSUM banks recycle quickly.
  4. Final: out = O^T-chunks.T @ Wo + bo (bo via pre-broadcast DVE add).
"""

import numpy as np

import concourse.bacc as bacc
import concourse.mybir as mybir
import concourse.tile as tile
from concourse import bass_utils
from concourse.masks import make_identity

F32 = mybir.dt.float32
BF16 = mybir.dt.bfloat16
EXP = mybir.ActivationFunctionType.Exp
COPY = mybir.ActivationFunctionType.Copy

B, S, D, H = 4, 2048, 1024, 16
SQ = 1024          # query rows per core
P = 128
MC = D // P        # 8 m-chunks (contraction of projections)
DKC = D // P       # 8 dk-chunks
KC = S // P        # 16 key chunks
SCALE = 1.0 / 32.0  # 1/sqrt(D_K)
N_CORES = 8

_CACHED_NC = None


def build_nc():
    nc = bacc.Bacc("TRN2", target_bir_lowering=False, debug=False,
                   num_devices=N_CORES)
    q_in = nc.dram_tensor("q_in", [SQ, D], F32, kind="ExternalInput")
    k_in = nc.dram_tensor("k_in", [S, D], F32, kind="ExternalInput")
    v_in = nc.dram_tensor("v_in", [S, D], F32, kind="ExternalInput")
    wq_d = nc.dram_tensor("wq", [D, D], F32, kind="ExternalInput")
    wk_d = nc.dram_tensor("wk", [D, D], F32, kind="ExternalInput")
    wv_d = nc.dram_tensor("wv", [D, D], F32, kind="ExternalInput")
    wo_d = nc.dram_tensor("wo", [D, D], F32, kind="ExternalInput")
    bq_d = nc.dram_tensor("bq", [D], F32, kind="ExternalInput")
    bk_d = nc.dram_tensor("bk", [D], F32, kind="ExternalInput")
    bv_d = nc.dram_tensor("bv", [D], F32, kind="ExternalInput")
    bo_d = nc.dram_tensor("bo", [D], F32, kind="ExternalInput")
    out_d = nc.dram_tensor("out", [SQ, D], F32, kind="ExternalOutput")

    with tile.TileContext(nc) as tc:
        _build_body(nc, tc, q_in, k_in, v_in, wq_d, wk_d, wv_d, wo_d,
                    bq_d, bk_d, bv_d, bo_d, out_d)
    nc.compile()
    return nc


def _head_stage(nc, x_d, n_rows, stg, ps_t, w_d, wpool, wtag, identb, dmae,
                evict, proj_group):
    """One head stage: DMA x row-chunks + weight chunks (both queues),
    cast x to bf16 on DVE (prefetched one group ahead), PE-transpose per
    group, then call proj_group(g, w_tiles) with the group's projections.

    evict(mm, g, psum) stores transposed [128, 512] blocks."""
    ngroups = n_rows // (4 * P)
    # DMA order per queue: first group's x chunks, all weight chunks, rest.
    raws = []
    for j in range(4):
        t = stg.tile([P, D], F32, tag="xin", bufs=3)
        dmae[j % 2].dma_start(t[:], x_d.ap()[j * P:(j + 1) * P, :])
        raws.append(t)
    wraws = []
    for mm in range(MC):
        raw = stg.tile([P, D], F32, tag="wraw", bufs=2)
        dmae[mm % 2].dma_start(raw[:], w_d.ap()[mm * P:(mm + 1) * P, :])
        wraws.append(raw)
    for r in range(4, 4 * ngroups):
        t = stg.tile([P, D], F32, tag="xin", bufs=3)
        dmae[r % 2].dma_start(t[:], x_d.ap()[r * P:(r + 1) * P, :])
        raws.append(t)

    def cast_group(g):
        # on ACT: the DVE is busy with projection evictions in the head,
        # while ACT only has the transpose-psum evictions
        rows = []
        for j in range(4):
            c = stg.tile([P, D], BF16, tag="xcast", bufs=8)
            nc.scalar.activation(c[:], raws[g * 4 + j][:], COPY)
            rows.append(c)
        return rows

    cur = cast_group(0)
    w_tiles = []
    for mm in range(MC):
        wt = wpool.tile([P, D], BF16, tag=f"{wtag}{mm}", name=f"w_{wtag}{mm}")
        nc.vector.tensor_copy(wt[:], wraws[mm][:])
        w_tiles.append(wt)

    for g in range(ngroups):
        rows, cur = cur, None
        for mm in range(MC):
            pst = ps_t.tile([P, 512], BF16, tag="pst", bufs=2)
            for j in range(4):
                nc.tensor.transpose(
                    pst[:, j * P:(j + 1) * P],
                    rows[j][:, mm * P:(mm + 1) * P], identb[:])
            evict(mm, g, pst)
            if mm == 1 and g + 1 < ngroups:
                cur = cast_group(g + 1)
        proj_group(g, w_tiles)


def _build_body(nc, tc, q_in, k_in, v_in, wq_d, wk_d, wv_d, wo_d,
                bq_d, bk_d, bv_d, bo_d, out_d):
    dmae = [nc.sync, nc.scalar]   # the two hwdge queues
    vcell = {}  # late-bound: wv tiles + vproj psum pool
    with (
        tc.tile_pool(name="const", bufs=1) as constp,
        tc.tile_pool(name="qtp", bufs=1) as qtp,
        tc.tile_pool(name="ktp", bufs=1) as ktp,
    ):
        ident = constp.tile([P, P], F32)
        make_identity(nc, ident[:])
        identb = constp.tile([P, P], BF16)
        nc.vector.tensor_copy(identb[:], ident[:])
        bq_t = constp.tile([P, MC], F32)
        nc.sync.dma_start(bq_t[:], bq_d.ap().rearrange("(c p) -> p c", p=P))
        bk_t = constp.tile([P, MC], F32)
        nc.sync.dma_start(bk_t[:], bk_d.ap().rearrange("(c p) -> p c", p=P))
        bv_t = constp.tile([P, MC], F32)
        nc.sync.dma_start(bv_t[:], bv_d.ap().rearrange("(c p) -> p c", p=P))
        bo_f = constp.tile([1, D], F32)
        nc.sync.dma_start(bo_f[:], bo_d.ap().unsqueeze(0))

        QT = [qtp.tile([P, SQ], BF16, tag=f"qt{i}", name=f"qt{i}")
              for i in range(DKC)]
        KT = [ktp.tile([P, S], BF16, tag=f"kt{i}", name=f"kt{i}")
              for i in range(DKC)]

        # ---------------- stage Q ----------------
        with (
            tc.tile_pool(name="stgq", bufs=1) as stg,
            tc.tile_pool(name="wq", bufs=1) as wpool,
            tc.tile_pool(name="xtq", bufs=1) as xtp,
            tc.tile_pool(name="psq_t", bufs=1, space="PSUM") as ps_t,
            tc.tile_pool(name="psq_p", bufs=2, space="PSUM") as ps_p,
        ):
            xqT = [xtp.tile([P, SQ], BF16, tag=f"xt{i}", name=f"xqt{i}")
                   for i in range(MC)]

            def evq(mm, g, pst):
                nc.scalar.activation(
                    xqT[mm][:, g * 512:(g + 1) * 512], pst[:], COPY)

            def projq(g, w_tiles):
                for dk in range(DKC):
                    ps = ps_p.tile([P, 512], F32, tag="pp")
                    for mm in range(MC):
                        nc.tensor.matmul(
                            ps[:], w_tiles[mm][:, dk * P:(dk + 1) * P],
                            xqT[mm][:, g * 512:(g + 1) * 512],
                            start=(mm == 0), stop=(mm == MC - 1))
                    nc.vector.tensor_scalar_add(
                        QT[dk][:, g * 512:(g + 1) * 512], ps[:],
                        bq_t[:, dk:dk + 1])

            _head_stage(nc, q_in, SQ, stg, ps_t, wq_d, wpool, "w", identb,
                        dmae, evq, projq)

        # ---------------- stage K ----------------
        with (
            tc.tile_pool(name="stgk", bufs=1) as stg,
            tc.tile_pool(name="wk", bufs=1) as wpool,
            tc.tile_pool(name="xtk", bufs=1) as xtp,
            tc.tile_pool(name="psk_t", bufs=1, space="PSUM") as ps_t,
            tc.tile_pool(name="psk_p", bufs=2, space="PSUM") as ps_p,
        ):
            xkT = [xtp.tile([P, S], BF16, tag=f"xt{i}", name=f"xkt{i}")
                   for i in range(MC)]

            def evk(mm, g, pst):
                nc.scalar.activation(
                    xkT[mm][:, g * 512:(g + 1) * 512], pst[:], COPY)

            def projk(g, w_tiles):
                for dk in range(DKC):
                    ps = ps_p.tile([P, 512], F32, tag="pp")
                    for mm in range(MC):
                        nc.tensor.matmul(
                            ps[:], w_tiles[mm][:, dk * P:(dk + 1) * P],
                            xkT[mm][:, g * 512:(g + 1) * 512],
                            start=(mm == 0), stop=(mm == MC - 1))
                    nc.vector.tensor_scalar_add(
                        KT[dk][:, g * 512:(g + 1) * 512], ps[:],
                        bk_t[:, dk:dk + 1])

            _head_stage(nc, k_in, S, stg, ps_t, wk_d, wpool, "w", identb,
                        dmae, evk, projk)

        # -------- persistent pools for V / attention / final --------
        DEXT = H * 65  # V_ext: 65 cols per head (64 V + ones)
        with (
            tc.tile_pool(name="xtv", bufs=1) as xvtp,
            tc.tile_pool(name="wv", bufs=1) as wvp,
            tc.tile_pool(name="vp", bufs=1) as vp,
            tc.tile_pool(name="otp", bufs=1) as otp,
        ):
            xvT = [xvtp.tile([P, S], BF16, tag=f"xt{i}", name=f"xvt{i}")
                   for i in range(MC)]
            V = [vp.tile([P, DEXT], BF16, tag=f"v{i}", name=f"v{i}")
                 for i in range(KC)]
            OT = [otp.tile([P, SQ], BF16, tag=f"ot{i}", name=f"ot{i}")
                  for i in range(DKC)]
            ones16 = constp.tile([P, H], BF16, name="ones16")
            nc.vector.memset(ones16[:], 1.0)

            def vproj_group(sc, nh):
                ps = vcell["ps"].tile([P, 512], F32, tag="m", bufs=1)
                for mm in range(MC):
                    nc.tensor.matmul(
                        ps[:], xvT[mm][:, sc * P:(sc + 1) * P],
                        vcell["wv"][mm][:, nh * 512:(nh + 1) * 512],
                        start=(mm == 0), stop=(mm == MC - 1))
                vx = V[sc].rearrange("p (h c) -> p h c", c=65)
                if nh == 0:
                    nc.vector.tensor_copy(
                        vx[:, :, 64:65],
                        ones16[:].rearrange("p (h c) -> p h c", c=1))
                nc.vector.tensor_copy(
                    vx[:, 8 * nh:8 * nh + 8, 0:64],
                    ps[:].rearrange("p (h c) -> p h c", c=64))

            # ---- stage V: transposes + nh=0 projections ----
            with (
                tc.tile_pool(name="stgv", bufs=1) as stg,
                tc.tile_pool(name="psv_t", bufs=1, space="PSUM") as ps_t,
                tc.tile_pool(name="psv_m", bufs=2, space="PSUM") as ps_vm,
            ):
                vcell["ps"] = ps_vm

                def evv(mm, g, pst):
                    nc.scalar.activation(
                        xvT[mm][:, g * 512:(g + 1) * 512], pst[:], COPY)

                def projv(g, w_tiles):
                    vcell["wv"] = w_tiles
                    for sc in range(4 * g, 4 * g + 4):
                        vproj_group(sc, 0)

                _head_stage(nc, v_in, S, stg, ps_t, wv_d, wvp, "w", identb,
                            dmae, evv, projv)

            # ---- attention + interleaved V-proj nh1 / final ----
            with (
                tc.tile_pool(name="wo", bufs=1) as wop,
                tc.tile_pool(name="ep", bufs=4) as ep,
                tc.tile_pool(name="rp", bufs=1) as rp,
                tc.tile_pool(name="bcp", bufs=1) as bcp,
                tc.tile_pool(name="fin", bufs=1) as finp,
                tc.tile_pool(name="ps_s", bufs=2, space="PSUM") as ps_s,
                tc.tile_pool(name="ps_pv", bufs=3, space="PSUM") as ps_pv,
                tc.tile_pool(name="ps_m", bufs=1, space="PSUM") as ps_m,
            ):
                vcell["ps"] = ps_m
                bo_bc = constp.tile([P, D], F32, name="bo_bc")
                nc.gpsimd.partition_broadcast(bo_bc[:, 0:512], bo_f[:, 0:512])
                nc.gpsimd.partition_broadcast(bo_bc[:, 512:1024],
                                              bo_f[:, 512:1024])

                wo_t = {}

                def wo_load(nh, dk):
                    raw = finp.tile([P, 512], F32, tag="wraw", bufs=1)
                    nc.scalar.dma_start(
                        raw[:],
                        wo_d.ap()[dk * P:(dk + 1) * P,
                                  nh * 512:(nh + 1) * 512])
                    wt = wop.tile([P, 512], BF16, tag=f"woh{nh}_{dk}",
                                  name=f"woh{nh}_{dk}")
                    nc.vector.tensor_copy(wt[:], raw[:])
                    wo_t[(nh, dk)] = wt

                def final_group(qt, nh, sc):
                    ss = slice(sc * P, (sc + 1) * P)
                    ns = slice(nh * 512, (nh + 1) * 512)
                    fps = ps_m.tile([P, 512], F32, tag="m")
                    for dk in range(DKC):
                        nc.tensor.matmul(
                            fps[:], OT[dk][:, ss], wo_t[(nh, dk)][:],
                            start=(dk == 0), stop=(dk == DKC - 1))
                    ob = finp.tile([P, 512], F32, tag="ob", bufs=2)
                    nc.vector.tensor_add(ob[:], fps[:], bo_bc[:, ns])
                    nc.sync.dma_start(out_d.ap()[ss, ns], ob[:])

                def make_norm(qs, pair, pv1, pv2):
                    def emit():
                        for hh, pvp in ((0, pv1), (1, pv2)):
                            psb = rp.tile([65, 512], F32, tag="psb", bufs=3,
                                          name="psb")
                            nc.vector.tensor_copy(psb[:], pvp[0:65, :])
                            sums = rp.tile([1, 512], F32, tag="sums", bufs=2,
                                           name="sums")
                            nc.gpsimd.tensor_copy(sums[:], psb[64:65, :])
                            nc.vector.reciprocal_approx_fast(sums[:], sums[:])
                            bc = bcp.tile([64, 512], F32, tag="bc", bufs=2,
                                          name="bc")
                            nc.gpsimd.partition_broadcast(bc[:], sums[:])
                            if hh == 0:
                                osl = OT[pair][0:64, qs]
                                nc.vector.tensor_mul(osl, psb[0:64, :], bc[:])
                                nc.vector.tensor_scalar_add(
                                    osl, osl, bv_t[0:64, pair:pair + 1])
                            else:
                                tmp = bcp.tile([64, 512], BF16, tag="tmp",
                                               bufs=2, name="tmp")
                                nc.vector.tensor_mul(tmp[:], psb[0:64, :],
                                                     bc[:])
                                osl = OT[pair][64:128, qs]
                                nc.sync.dma_start(osl, tmp[:])
                                nc.vector.tensor_scalar_add(
                                    osl, osl, bv_t[64:128, pair:pair + 1])
                    return emit

                def make_pv(pv1, pv2, e1, e2, c1, c2, k2):
                    first = k2 == 0
                    last = k2 == KC // 2 - 1

                    def emit():
                        nc.tensor.matmul(
                            pv1[0:65, :], V[2 * k2][:, c1:c1 + 65],
                            e1[:, 0:512], start=first, stop=False)
                        nc.tensor.matmul(
                            pv2[0:65, :], V[2 * k2][:, c2:c2 + 65],
                            e2[:, 0:512], start=first, stop=False)
                        nc.tensor.matmul(
                            pv1[0:65, :], V[2 * k2 + 1][:, c1:c1 + 65],
                            e1[:, 512:1024], start=False, stop=last)
                        nc.tensor.matmul(
                            pv2[0:65, :], V[2 * k2 + 1][:, c2:c2 + 65],
                            e2[:, 512:1024], start=False, stop=last)
                    return emit

                # interleaved (qt, pair) block order: qt0 leads by two pairs
                blocks = [(0, 0), (0, 1)]
                for p in range(6):
                    blocks += [(1, p), (0, p + 2)]
                blocks += [(1, 6), (1, 7)]

                # filler schedule over the 128 iterations
                fillers = {}
                for s in range(16):     # V nh1, done before qt0-pair4 (it 56)
                    fillers.setdefault(16 + (s * 5) // 2, []).append(
                        lambda sc=s: vproj_group(sc, 1))
                for i in range(16):     # wo loads (no PE work)
                    nh, dk = divmod(i, 8)
                    fillers.setdefault(30 + i, []).append(
                        lambda n=nh, d=dk: wo_load(n, d))
                for i in range(8):      # final(qt0) during the B6/B7 tail
                    nh, sc = divmod(i, 4)
                    fillers.setdefault(112 + 2 * i, []).append(
                        lambda n=nh, s=sc: final_group(0, n, s))

                pend_pv = None
                pend_norm = None
                it = 0
                for qt, pair in blocks:
                    qs = slice(qt * 512, (qt + 1) * 512)
                    pv1 = ps_pv.tile([P, 512], F32, tag="pv")
                    pv2 = ps_pv.tile([P, 512], F32, tag="pv")
                    c1 = (2 * pair) * 65
                    c2 = (2 * pair + 1) * 65
                    for k2 in range(KC // 2):
                        ka = slice(2 * k2 * P, (2 * k2 + 1) * P)
                        kb = slice((2 * k2 + 1) * P, (2 * k2 + 2) * P)
                        s1 = ps_s.tile([P, 1024], F32, tag="sc")
                        s2 = ps_s.tile([P, 1024], F32, tag="sc")
                        nc.tensor.matmul(
                            s1[:, 0:512], KT[pair][0:64, ka],
                            QT[pair][0:64, qs], start=True, stop=True,
                            tile_position=(0, 0))
                        nc.tensor.matmul(
                            s2[:, 0:512], KT[pair][64:128, ka],
                            QT[pair][64:128, qs], start=True, stop=True,
                            tile_position=(64, 0))
                        nc.tensor.matmul(
                            s1[:, 512:1024], KT[pair][0:64, kb],
                            QT[pair][0:64, qs], start=True, stop=True,
                            tile_position=(0, 0))
                        nc.tensor.matmul(
                            s2[:, 512:1024], KT[pair][64:128, kb],
                            QT[pair][64:128, qs], start=True, stop=True,
                            tile_position=(64, 0))
                        e1 = ep.tile([P, 1024], BF16, tag="e")
                        e2 = ep.tile([P, 1024], BF16, tag="e")
                        nc.scalar.activation(e1[:], s1[:], EXP, scale=SCALE)
                        nc.scalar.activation(e2[:], s2[:], EXP, scale=SCALE)
                        if pend_pv is not None:
                            pend_pv()
                            pend_pv = None
                        if pend_norm is not None:
                            # after the prev block's last PV (flushed just
                            # above at k2==0), before its psum bufs rotate
                            # into reuse by this block's PV
                            pend_norm()
                            pend_norm = None
                        pend_pv = make_pv(pv1, pv2, e1, e2, c1, c2, k2)
                        if k2 == KC // 2 - 1:
                            pend_norm_next = make_norm(qs, pair, pv1, pv2)
                        for f in fillers.get(it, ()):
                            f()
                        it += 1
                    pend_norm = pend_norm_next
                # drain the pipeline
                if pend_pv is not None:
                    pend_pv()
                if pend_norm is not None:
                    pend_norm()
                # final projection for qt=1
                for nh in range(2):
                    for sc in range(4, 8):
                        final_group(1, nh, sc)


def get_nc():
    global _CACHED_NC
    if _CACHED_NC is None:
        _CACHED_NC = build_nc()
    return _CACHED_NC


def run(inputs, **kwargs):
    """Run on 8 cores; returns (full_output, BassKernelResults)."""
    nc = get_nc()
    queries = np.ascontiguousarray(np.asarray(inputs["queries"], np.float32))
    keys = np.ascontiguousarray(np.asarray(inputs["keys"], np.float32))
    values = np.ascontiguousarray(np.asarray(inputs["values"], np.float32))
    base = {
        "wq": np.ascontiguousarray(np.asarray(inputs["Wq"], np.float32)),
        "wk": np.ascontiguousarray(np.asarray(inputs["Wk"], np.float32)),
        "wv": np.ascontiguousarray(np.asarray(inputs["Wv"], np.float32)),
        "wo": np.ascontiguousarray(np.asarray(inputs["Wo"], np.float32)),
        "bq": np.ascontiguousarray(np.asarray(inputs["bq"], np.float32)),
        "bk": np.ascontiguousarray(np.asarray(inputs["bk"], np.float32)),
        "bv": np.ascontiguousarray(np.asarray(inputs["bv"], np.float32)),
        "bo": np.ascontiguousarray(np.asarray(inputs["bo"], np.float32)),
    }
    in_maps = []
    for c in range(N_CORES):
        b, qh = c // 2, c % 2
        m = dict(base)
        m["q_in"] = np.ascontiguousarray(queries[b, qh * SQ:(qh + 1) * SQ])
        m["k_in"] = keys[b]
        m["v_in"] = values[b]
        in_maps.append(m)
    res = bass_utils.run_bass_kernel_spmd(
        nc, in_maps, core_ids=list(range(N_CORES)), **kwargs)
    out = np.empty((B, S, D), np.float32)
    for c in range(N_CORES):
        b, qh = c // 2, c % 2
        out[b, qh * SQ:(qh + 1) * SQ] = res.results[c]["out"]
    return out, res


def kernel(**inputs):
    out, _ = run(inputs)
    return out


if __name__ == "__main__":
    rng = np.random.default_rng(0)
    ins = {
        "queries": rng.standard_normal((B, S, D), dtype=np.float32),
        "keys": rng.standard_normal((B, S, D), dtype=np.float32),
        "values": rng.standard_normal((B, S, D), dtype=np.float32),
        "Wq": (rng.standard_normal((D, D), dtype=np.float32) / 32),
        "bq": np.zeros(D, np.float32),
        "Wk": (rng.standard_normal((D, D), dtype=np.float32) / 32),
        "bk": np.zeros(D, np.float32),
        "Wv": (rng.standard_normal((D, D), dtype=np.float32) / 32),
        "bv": np.zeros(D, np.float32),
        "Wo": (rng.standard_normal((D, D), dtype=np.float32) / 32),
        "bo": np.zeros(D, np.float32),
    }
    out = kernel(**ins)
    print("out", out.shape, out.dtype, np.abs(out).mean())


# revision 18
# speedup vs baseline: 1.1717x; 1.0145x over previous
"""Multi-head attention (B=4, S=2048, D=1024, H=16) on 8 TRN2 NeuronCores.

Sharding: core c handles batch b = c//2 and query-half qh = c%2 (1024 query
rows), with K/V projection for its batch replicated across the 2 cores that
share the batch. Zero inter-core communication; host just slices inputs and
concatenates outputs.

Per-core dataflow (all matmuls and transposes bf16):
  1. Head stages (Q, K, V): per 512-row group, DMA -> DVE cast to bf16 ->
     PE transpose -> projection matmuls, interleaved so the PE stays dense
     behind the DMA stream (keeps the HAM clock-gate warm).
  2. Attention main loop over interleaved (qt, pair) blocks
     [A0 A1 B0 A2 B1 ... A7 B6 B7], software-pipelined: scores(k2) issued
     before PV(k2-1) so the PE never stalls on the exp; V-projection for
     heads 8-15, Wo loads and the final projection for the finished q-half
     are spread as PE filler across all iterations.
  3. Softmax sums ride the PV matmul as a 65th V column; normalization on
     DVE/GPSIMD with a fast psum eviction so PSUM banks recycle quickly.
  4. Final: out = O^T-chunks.T @ Wo + bo (bo via pre-broadcast DVE add).
"""

import numpy as np

import concourse.bacc as bacc
import concourse.mybir as mybir
import concourse.tile as tile
from concourse import bass_utils
from concourse.masks import make_identity

F32 = mybir.dt.float32
BF16 = mybir.dt.bfloat16
EXP = mybir.ActivationFunctionType.Exp
COPY = mybir.ActivationFunctionType.Copy

B, S, D, H = 4, 2048, 1024, 16
SQ = 1024          # query rows per core
P = 128
MC = D // P        # 8 m-chunks (contraction of projections)
DKC = D // P       # 8 dk-chunks
KC = S // P        # 16 key chunks
SCALE = 1.0 / 32.0  # 1/sqrt(D_K)
N_CORES = 8

_CACHED_NC = None


def build_nc():
    nc = bacc.Bacc("TRN2", target_bir_lowering=False, debug=False,
                   num_devices=N_CORES)
    q_in = nc.dram_tensor("q_in", [SQ, D], F32, kind="ExternalInput")
    k_in = nc.dram_tensor("k_in", [S, D], F32, kind="ExternalInput")
    v_in = nc.dram_tensor("v_in", [S, D], F32, kind="ExternalInput")
    wq_d = nc.dram_tensor("wq", [D, D], F32, kind="ExternalInput")
    wk_d = nc.dram_tensor("wk", [D, D], F32, kind="ExternalInput")
    wv_d = nc.dram_tensor("wv", [D, D], F32, kind="ExternalInput")
    wo_d = nc.dram_tensor("wo", [D, D], F32, kind="ExternalInput")
    bq_d = nc.dram_tensor("bq", [D], F32, kind="ExternalInput")
    bk_d = nc.dram_tensor("bk", [D], F32, kind="ExternalInput")
    bv_d = nc.dram_tensor("bv", [D], F32, kind="ExternalInput")
    bo_d = nc.dram_tensor("bo", [D], F32, kind="ExternalInput")
    out_d = nc.dram_tensor("out", [SQ, D], F32, kind="ExternalOutput")

    with tile.TileContext(nc) as tc:
        _build_body(nc, tc, q_in, k_in, v_in, wq_d, wk_d, wv_d, wo_d,
                    bq_d, bk_d, bv_d, bo_d, out_d)
    nc.compile()
    return nc


def _head_stage(nc, x_d, n_rows, stg, ps_t, w_d, wpool, wtag, identb, dmae,
                evict, proj_group, w_cols=D):
    """One head stage: DMA x row-chunks + weight chunks (both queues),
    cast x to bf16 on DVE (prefetched one group ahead), PE-transpose per
    group, then call proj_group(g, w_tiles) with the group's projections.

    evict(mm, g, psum) stores transposed [128, 512] blocks."""
    ngroups = n_rows // (4 * P)
    # DMA order per queue: first group's x chunks, all weight chunks, rest.
    raws = []
    for j in range(4):
        t = stg.tile([P, D], F32, tag="xin", bufs=2)
        dmae[j % 2].dma_start(t[:], x_d.ap()[j * P:(j + 1) * P, :])
        raws.append(t)
    wraws = []
    for mm in range(MC):
        raw = stg.tile([P, w_cols], F32, tag="wraw", bufs=2)
        dmae[mm % 2].dma_start(raw[:],
                              w_d.ap()[mm * P:(mm + 1) * P, 0:w_cols])
        wraws.append(raw)
    for r in range(4, 4 * ngroups):
        t = stg.tile([P, D], F32, tag="xin", bufs=2)
        dmae[r % 2].dma_start(t[:], x_d.ap()[r * P:(r + 1) * P, :])
        raws.append(t)

    def cast_group(g):
        # on ACT: the DVE is busy with projection evictions in the head,
        # while ACT only has the transpose-psum evictions
        rows = []
        for j in range(4):
            c = stg.tile([P, D], BF16, tag="xcast", bufs=6)
            nc.scalar.activation(c[:], raws[g * 4 + j][:], COPY)
            rows.append(c)
        return rows

    cur = cast_group(0)
    w_tiles = []
    for mm in range(MC):
        wt = wpool.tile([P, w_cols], BF16, tag=f"{wtag}{mm}",
                        name=f"w_{wtag}{mm}")
        nc.vector.tensor_copy(wt[:], wraws[mm][:])
        w_tiles.append(wt)

    for g in range(ngroups):
        rows, cur = cur, None
        for mm in range(MC):
            pst = ps_t.tile([P, 512], BF16, tag="pst", bufs=2)
            for j in range(4):
                nc.tensor.transpose(
                    pst[:, j * P:(j + 1) * P],
                    rows[j][:, mm * P:(mm + 1) * P], identb[:])
            evict(mm, g, pst)
            if mm == 1 and g + 1 < ngroups:
                cur = cast_group(g + 1)
        proj_group(g, w_tiles)


def _build_body(nc, tc, q_in, k_in, v_in, wq_d, wk_d, wv_d, wo_d,
                bq_d, bk_d, bv_d, bo_d, out_d):
    dmae = [nc.sync, nc.scalar]   # the two hwdge queues
    vcell = {}  # late-bound: wv tiles + vproj psum pool
    with (
        tc.tile_pool(name="const", bufs=1) as constp,
        tc.tile_pool(name="qtp", bufs=1) as qtp,
        tc.tile_pool(name="ktp", bufs=1) as ktp,
    ):
        ident = constp.tile([P, P], F32)
        make_identity(nc, ident[:])
        identb = constp.tile([P, P], BF16)
        nc.vector.tensor_copy(identb[:], ident[:])
        # biases: contiguous [8,128] loads (a (c p)->p c DMA would emit 1024
        # 4-byte descriptors at the head of the queue); PE-transposed below
        braw = constp.tile([MC, 3 * P], F32, name="braw")
        nc.scalar.dma_start(braw[:, 0:P],
                            bq_d.ap().rearrange("(c p) -> c p", p=P))
        nc.scalar.dma_start(braw[:, P:2 * P],
                            bk_d.ap().rearrange("(c p) -> c p", p=P))
        nc.scalar.dma_start(braw[:, 2 * P:3 * P],
                            bv_d.ap().rearrange("(c p) -> c p", p=P))
        bo_f = constp.tile([1, D], F32)
        nc.scalar.dma_start(bo_f[:], bo_d.ap().unsqueeze(0))
        bqkv_t = constp.tile([P, 3 * MC], F32, name="bqkv_t")
        bq_t = bqkv_t[:, 0:MC]
        bk_t = bqkv_t[:, MC:2 * MC]
        bv_t = bqkv_t[:, 2 * MC:3 * MC]

        QT = [qtp.tile([P, SQ], BF16, tag=f"qt{i}", name=f"qt{i}")
              for i in range(DKC)]
        KT = [ktp.tile([P, S], BF16, tag=f"kt{i}", name=f"kt{i}")
              for i in range(DKC)]

        # ---------------- stage Q ----------------
        with (
            tc.tile_pool(name="stgq", bufs=1) as stg,
            tc.tile_pool(name="wq", bufs=1) as wpool,
            tc.tile_pool(name="xtq", bufs=1) as xtp,
            tc.tile_pool(name="psq_t", bufs=1, space="PSUM") as ps_t,
            tc.tile_pool(name="psq_p", bufs=2, space="PSUM") as ps_p,
        ):
            xqT = [xtp.tile([P, SQ], BF16, tag=f"xt{i}", name=f"xqt{i}")
                   for i in range(MC)]

            def evq(mm, g, pst):
                nc.scalar.activation(
                    xqT[mm][:, g * 512:(g + 1) * 512], pst[:], COPY)

            def projq(g, w_tiles):
                for dk in range(DKC):
                    ps = ps_p.tile([P, 512], F32, tag="pp")
                    for mm in range(MC):
                        nc.tensor.matmul(
                            ps[:], w_tiles[mm][:, dk * P:(dk + 1) * P],
                            xqT[mm][:, g * 512:(g + 1) * 512],
                            start=(mm == 0), stop=(mm == MC - 1))
                    nc.vector.tensor_scalar_add(
                        QT[dk][:, g * 512:(g + 1) * 512], ps[:],
                        bq_t[:, dk:dk + 1])

            bps = ps_t.tile([P, 512], F32, tag="bps", bufs=1)
            for i in range(3):
                nc.tensor.transpose(bps[:, i * MC:(i + 1) * MC],
                                    braw[:, i * P:(i + 1) * P], ident[0:MC,
                                                                      0:MC])
            nc.vector.tensor_copy(bqkv_t[:], bps[:, 0:3 * MC])

            _head_stage(nc, q_in, SQ, stg, ps_t, wq_d, wpool, "w", identb,
                        dmae, evq, projq)


        # -------- persistent pools for deferred K-proj / V --------
        DEXT = H * 65  # V_ext: 65 cols per head (64 V + ones)
        with (
            tc.tile_pool(name="xtk", bufs=1) as xktp,
            tc.tile_pool(name="vp", bufs=1) as vp,
        ):
            xkT = [xktp.tile([P, S], BF16, tag=f"xt{i}", name=f"xkt{i}")
                   for i in range(MC)]
            V = [vp.tile([P, DEXT], BF16, tag=f"v{i}", name=f"v{i}")
                 for i in range(KC)]
            ones16 = constp.tile([P, H], BF16, name="ones16")
            nc.vector.memset(ones16[:], 1.0)

            # ---------------- stage K (projects dk 0-3; 4-7 deferred) ----
            with (
                tc.tile_pool(name="stgk", bufs=1) as stg,
                tc.tile_pool(name="wk", bufs=1) as wpool,
                tc.tile_pool(name="psk_t", bufs=1, space="PSUM") as ps_t,
                tc.tile_pool(name="psk_p", bufs=2, space="PSUM") as ps_p,
            ):
                def evk(mm, g, pst):
                    nc.scalar.activation(
                        xkT[mm][:, g * 512:(g + 1) * 512], pst[:], COPY)

                def projk(g, w_tiles):
                    for dk in range(DKC // 2):
                        ps = ps_p.tile([P, 512], F32, tag="pp")
                        for mm in range(MC):
                            nc.tensor.matmul(
                                ps[:], w_tiles[mm][:, dk * P:(dk + 1) * P],
                                xkT[mm][:, g * 512:(g + 1) * 512],
                                start=(mm == 0), stop=(mm == MC - 1))
                        nc.vector.tensor_scalar_add(
                            KT[dk][:, g * 512:(g + 1) * 512], ps[:],
                            bk_t[:, dk:dk + 1])

                _head_stage(nc, k_in, S, stg, ps_t, wk_d, wpool, "w",
                            identb, dmae, evk, projk, w_cols=512)

            # ---- stage V: transposes + full projections ----
            with (
                tc.tile_pool(name="stgv", bufs=1) as stg,
                tc.tile_pool(name="wv", bufs=1) as wvp,
                tc.tile_pool(name="xtv", bufs=1) as xvtp,
                tc.tile_pool(name="psv_t", bufs=1, space="PSUM") as ps_t,
                tc.tile_pool(name="psv_m", bufs=2, space="PSUM") as ps_vm,
            ):
                xvT = [xvtp.tile([P, S], BF16, tag=f"xt{i}", name=f"xvt{i}")
                       for i in range(MC)]

                def evv(mm, g, pst):
                    nc.scalar.activation(
                        xvT[mm][:, g * 512:(g + 1) * 512], pst[:], COPY)

                def vproj_group(sc, nh, w_tiles):
                    ps = ps_vm.tile([P, 512], F32, tag="m")
                    for mm in range(MC):
                        nc.tensor.matmul(
                            ps[:], xvT[mm][:, sc * P:(sc + 1) * P],
                            w_tiles[mm][:, nh * 512:(nh + 1) * 512],
                            start=(mm == 0), stop=(mm == MC - 1))
                    vx = V[sc].rearrange("p (h c) -> p h c", c=65)
                    if nh == 0:
                        nc.vector.tensor_copy(
                            vx[:, :, 64:65],
                            ones16[:].rearrange("p (h c) -> p h c", c=1))
                    nc.vector.tensor_copy(
                        vx[:, 8 * nh:8 * nh + 8, 0:64],
                        ps[:].rearrange("p (h c) -> p h c", c=64))

                def projv(g, w_tiles):
                    for sc in range(4 * g, 4 * g + 4):
                        vproj_group(sc, 0, w_tiles)
                        vproj_group(sc, 1, w_tiles)

                _head_stage(nc, v_in, S, stg, ps_t, wv_d, wvp, "w", identb,
                            dmae, evv, projv)

            # ---- attention + deferred K-proj dk4-7 / final ----
            with (
                tc.tile_pool(name="otp", bufs=1) as otp,
                tc.tile_pool(name="wo", bufs=1) as wop,
                tc.tile_pool(name="wk2", bufs=1) as wk2p,
                tc.tile_pool(name="ep", bufs=6) as ep,
                tc.tile_pool(name="rp", bufs=1) as rp,
                tc.tile_pool(name="bcp", bufs=1) as bcp,
                tc.tile_pool(name="fin", bufs=1) as finp,
                tc.tile_pool(name="ps_s", bufs=2, space="PSUM") as ps_s,
                tc.tile_pool(name="ps_pv", bufs=3, space="PSUM") as ps_pv,
                tc.tile_pool(name="ps_m", bufs=1, space="PSUM") as ps_m,
            ):
                OT = [otp.tile([P, SQ], BF16, tag=f"ot{i}", name=f"ot{i}")
                      for i in range(DKC)]
                bo_bc = constp.tile([P, D], F32, name="bo_bc")
                nc.gpsimd.partition_broadcast(bo_bc[:, 0:512], bo_f[:, 0:512])
                nc.gpsimd.partition_broadcast(bo_bc[:, 512:1024],
                                              bo_f[:, 512:1024])

                wo_t = {}

                def wo_load(nh, dk):
                    raw = finp.tile([P, 512], F32, tag="wraw", bufs=1)
                    nc.scalar.dma_start(
                        raw[:],
                        wo_d.ap()[dk * P:(dk + 1) * P,
                                  nh * 512:(nh + 1) * 512])
                    wt = wop.tile([P, 512], BF16, tag=f"woh{nh}_{dk}",
                                  name=f"woh{nh}_{dk}")
                    nc.vector.tensor_copy(wt[:], raw[:])
                    wo_t[(nh, dk)] = wt

                wk2 = {}

                def wk2_load(mm):
                    raw = finp.tile([P, 512], F32, tag="wraw", bufs=1)
                    nc.scalar.dma_start(
                        raw[:], wk_d.ap()[mm * P:(mm + 1) * P, 512:1024])
                    wt = wk2p.tile([P, 512], BF16, tag=f"wk2_{mm}",
                                   name=f"wk2_{mm}")
                    nc.vector.tensor_copy(wt[:], raw[:])
                    wk2[mm] = wt

                def kproj2(dk, g):
                    # deferred K projection for dk 4-7 (wk cols 512:1024)
                    ps = ps_m.tile([P, 512], F32, tag="m")
                    for mm in range(MC):
                        nc.tensor.matmul(
                            ps[:], wk2[mm][:, (dk - 4) * P:(dk - 3) * P],
                            xkT[mm][:, g * 512:(g + 1) * 512],
                            start=(mm == 0), stop=(mm == MC - 1))
                    nc.vector.tensor_scalar_add(
                        KT[dk][:, g * 512:(g + 1) * 512], ps[:],
                        bk_t[:, dk:dk + 1])

                def final_group(qt, nh, sc):
                    ss = slice(sc * P, (sc + 1) * P)
                    ns = slice(nh * 512, (nh + 1) * 512)
                    fps = ps_m.tile([P, 512], F32, tag="m")
                    for dk in range(DKC):
                        nc.tensor.matmul(
                            fps[:], OT[dk][:, ss], wo_t[(nh, dk)][:],
                            start=(dk == 0), stop=(dk == DKC - 1))
                    ob = finp.tile([P, 512], F32, tag="ob", bufs=2)
                    nc.vector.tensor_add(ob[:], fps[:], bo_bc[:, ns])
                    nc.sync.dma_start(out_d.ap()[ss, ns], ob[:])

                def make_norm(qs, pair, pv1, pv2):
                    def emit():
                        for hh, pvp in ((0, pv1), (1, pv2)):
                            psb = rp.tile([65, 512], F32, tag="psb", bufs=3,
                                          name="psb")
                            nc.vector.tensor_copy(psb[:], pvp[0:65, :])
                            sums = rp.tile([1, 512], F32, tag="sums", bufs=2,
                                           name="sums")
                            nc.gpsimd.tensor_copy(sums[:], psb[64:65, :])
                            nc.vector.reciprocal_approx_fast(sums[:], sums[:])
                            bc = bcp.tile([64, 512], F32, tag="bc", bufs=2,
                                          name="bc")
                            nc.gpsimd.partition_broadcast(bc[:], sums[:])
                            if hh == 0:
                                osl = OT[pair][0:64, qs]
                                nc.vector.tensor_mul(osl, psb[0:64, :], bc[:])
                                nc.vector.tensor_scalar_add(
                                    osl, osl, bv_t[0:64, pair:pair + 1])
                            else:
                                tmp = bcp.tile([64, 512], BF16, tag="tmp",
                                               bufs=2, name="tmp")
                                nc.vector.tensor_mul(tmp[:], psb[0:64, :],
                                                     bc[:])
                                osl = OT[pair][64:128, qs]
                                nc.sync.dma_start(osl, tmp[:])
                                nc.vector.tensor_scalar_add(
                                    osl, osl, bv_t[64:128, pair:pair + 1])
                    return emit

                def make_pv(pv1, pv2, e1, e2, c1, c2, k2):
                    first = k2 == 0
                    last = k2 == KC // 2 - 1

                    def emit():
                        nc.tensor.matmul(
                            pv1[0:65, :], V[2 * k2][:, c1:c1 + 65],
                            e1[:, 0:512], start=first, stop=False)
                        nc.tensor.matmul(
                            pv2[0:65, :], V[2 * k2][:, c2:c2 + 65],
                            e2[:, 0:512], start=first, stop=False)
                        nc.tensor.matmul(
                            pv1[0:65, :], V[2 * k2 + 1][:, c1:c1 + 65],
                            e1[:, 512:1024], start=False, stop=last)
                        nc.tensor.matmul(
                            pv2[0:65, :], V[2 * k2 + 1][:, c2:c2 + 65],
                            e2[:, 512:1024], start=False, stop=last)
                    return emit

                # interleaved (qt, pair) block order: qt0 leads by two pairs
                blocks = [(0, 0), (0, 1)]
                for p in range(6):
                    blocks += [(1, p), (0, p + 2)]
                blocks += [(1, 6), (1, 7)]

                # filler schedule over the 128 iterations
                fillers = {}
                for mm in range(MC):    # wk cols 512:1024 reload (no PE)
                    fillers.setdefault(1 + mm, []).append(
                        lambda m=mm: wk2_load(m))
                for i in range(16):     # wo loads (no PE work)
                    nh, dk = divmod(i, 8)
                    fillers.setdefault(9 + i, []).append(
                        lambda n=nh, d=dk: wo_load(n, d))
                # deferred K-proj: KT[4] by it 56, [5] by 72, [6] by 88,
                # [7] by 104; spread for PE density through mid-attention
                kp_slots = [10, 16, 22, 28, 34, 40, 46, 52,
                            58, 64, 70, 76, 82, 87, 92, 97]
                for i, slot in enumerate(kp_slots):
                    dk, g = 4 + i // 4, i % 4
                    fillers.setdefault(slot, []).append(
                        lambda d=dk, gg=g: kproj2(d, gg))
                for i in range(8):      # final(qt0) during the B6/B7 tail
                    nh, sc = divmod(i, 4)
                    fillers.setdefault(112 + 2 * i, []).append(
                        lambda n=nh, s=sc: final_group(0, n, s))

                pend_pv = None
                pend_norm = None
                it = 0
                for qt, pair in blocks:
                    qs = slice(qt * 512, (qt + 1) * 512)
                    pv1 = ps_pv.tile([P, 512], F32, tag="pv")
                    pv2 = ps_pv.tile([P, 512], F32, tag="pv")
                    c1 = (2 * pair) * 65
                    c2 = (2 * pair + 1) * 65
                    for k2 in range(KC // 2):
                        ka = slice(2 * k2 * P, (2 * k2 + 1) * P)
                        kb = slice((2 * k2 + 1) * P, (2 * k2 + 2) * P)
                        s1 = ps_s.tile([P, 1024], F32, tag="sc")
                        s2 = ps_s.tile([P, 1024], F32, tag="sc")
                        nc.tensor.matmul(
                            s1[:, 0:512], KT[pair][0:64, ka],
                            QT[pair][0:64, qs], start=True, stop=True,
                            tile_position=(0, 0))
                        nc.tensor.matmul(
                            s2[:, 0:512], KT[pair][64:128, ka],
                            QT[pair][64:128, qs], start=True, stop=True,
                            tile_position=(64, 0))
                        nc.tensor.matmul(
                            s1[:, 512:1024], KT[pair][0:64, kb],
                            QT[pair][0:64, qs], start=True, stop=True,
                            tile_position=(0, 0))
                        nc.tensor.matmul(
                            s2[:, 512:1024], KT[pair][64:128, kb],
                            QT[pair][64:128, qs], start=True, stop=True,
                            tile_position=(64, 0))
                        e1 = ep.tile([P, 1024], BF16, tag="e")
                        e2 = ep.tile([P, 1024], BF16, tag="e")
                        nc.scalar.activation(e1[:], s1[:], EXP, scale=SCALE)
                        nc.scalar.activation(e2[:], s2[:], EXP, scale=SCALE)
                        if pend_pv is not None:
                            pend_pv()
                            pend_pv = None
                        if pend_norm is not None:
                            # after the prev block's last PV (flushed just
                            # above at k2==0), before its psum bufs rotate
                            # into reuse by this block's PV
                            pend_norm()
                            pend_norm = None
                        pend_pv = make_pv(pv1, pv2, e1, e2, c1, c2, k2)
                        if k2 == KC // 2 - 1:
                            pend_norm_next = make_norm(qs, pair, pv1, pv2)
                        for f in fillers.get(it, ()):
                            f()
                        it += 1
                    pend_norm = pend_norm_next
                # drain the pipeline
                if pend_pv is not None:
                    pend_pv()
                if pend_norm is not None:
                    pend_norm()
                # final projection for qt=1
                for nh in range(2):
                    for sc in range(4, 8):
                        final_group(1, nh, sc)


def get_nc():
    global _CACHED_NC
    if _CACHED_NC is None:
        _CACHED_NC = build_nc()
    return _CACHED_NC


def run(inputs, **kwargs):
    """Run on 8 cores; returns (full_output, BassKernelResults)."""
    nc = get_nc()
    queries = np.ascontiguousarray(np.asarray(inputs["queries"], np.float32))
    keys = np.ascontiguousarray(np.asarray(inputs["keys"], np.float32))
    values = np.ascontiguousarray(np.asarray(inputs["values"], np.float32))
    base = {
        "wq": np.ascontiguousarray(np.asarray(inputs["Wq"], np.float32)),
        "wk": np.ascontiguousarray(np.asarray(inputs["Wk"], np.float32)),
        "wv": np.ascontiguousarray(np.asarray(inputs["Wv"], np.float32)),
        "wo": np.ascontiguousarray(np.asarray(inputs["Wo"], np.float32)),
        "bq": np.ascontiguousarray(np.asarray(inputs["bq"], np.float32)),
        "bk": np.ascontiguousarray(np.asarray(inputs["bk"], np.float32)),
        "bv": np.ascontiguousarray(np.asarray(inputs["bv"], np.float32)),
        "bo": np.ascontiguousarray(np.asarray(inputs["bo"], np.float32)),
    }
    in_maps = []
    for c in range(N_CORES):
        b, qh = c // 2, c % 2
        m = dict(base)
        m["q_in"] = np.ascontiguousarray(queries[b, qh * SQ:(qh + 1) * SQ])
        m["k_in"] = keys[b]
        m["v_in"] = values[b]
        in_maps.append(m)
    res = bass_utils.run_bass_kernel_spmd(
        nc, in_maps, core_ids=list(range(N_CORES)), **kwargs)
    out = np.empty((B, S, D), np.float32)
    for c in range(N_CORES):
        b, qh = c // 2, c % 2
        out[b, qh * SQ:(qh + 1) * SQ] = res.results[c]["out"]
    return out, res


def kernel(**inputs):
    out, _ = run(inputs)
    return out


if __name__ == "__main__":
    rng = np.random.default_rng(0)
    ins = {
        "queries": rng.standard_normal((B, S, D), dtype=np.float32),
        "keys": rng.standard_normal((B, S, D), dtype=np.float32),
        "values": rng.standard_normal((B, S, D), dtype=np.float32),
        "Wq": (rng.standard_normal((D, D), dtype=np.float32) / 32),
        "bq": np.zeros(D, np.float32),
        "Wk": (rng.standard_normal((D, D), dtype=np.float32) / 32),
        "bk": np.zeros(D, np.float32),
        "Wv": (rng.standard_normal((D, D), dtype=np.float32) / 32),
        "bv": np.zeros(D, np.float32),
        "Wo": (rng.standard_normal((D, D), dtype=np.float32) / 32),
        "bo": np.zeros(D, np.float32),
    }
    out = kernel(**ins)
    print("out", out.shape, out.dtype, np.abs(out).mean())
